# revision 11
# baseline (speedup 1.0000x reference)
"""Trainium2 Bass kernel for the masked-SST CNN encoder.

Pipeline per image (128x128 fp32 field, ~30% NaN):
  1. Jacobi NaN-filling: init = (row_nanmean+col_nanmean)/2 at masked px,
     then 100 iterations of 4-neighbor averaging (reflect padding) applied
     only at masked pixels.
  2. LayerNorm over the whole 128x128 patch (*ln_w + ln_b).
  3. conv5x5 s2 p2 -> 16ch, gelu(tanh); conv3x3 s2 p1 -> 32ch, gelu;
     conv3x3 s2 p1 -> 64ch, gelu; global avg pool; linear -> 5.
  4. mu/sigma assembly (host-side; O(B*5) work).

Distribution: pure data parallel, batch 1024 = 128 images per core x 8 cores.

Device strategy (per core, field SBUF-resident):
  - F stored as [128 part = row, img, col]. Vertical neighbor sums are a
    tridiagonal matmul on TensorE, horizontal sums are shifted-AP adds on
    VectorE, combined for free via PSUM accumulation; the masked update is
    one copy_predicated per block.
  - Convs contract over the partition dim with host-precomputed banded
    weight matrices; kernel-x taps are column-shifted stride-2 rhs APs
    accumulated in PSUM; GELU runs on ScalarE during the PSUM->SBUF drain.
"""

import numpy as np
from contextlib import ExitStack

import concourse.bass as bass
import concourse.bacc as bacc
import concourse.tile as tile
import concourse.mybir as mybir
from concourse import bass_utils

F32 = mybir.dt.float32
U8 = mybir.dt.uint8
AF = mybir.ActivationFunctionType
OP = mybir.AluOpType

B_FULL, M = 1024, 128
NCORES = 8
IMG = B_FULL // NCORES          # images per core
NUM_ITERS = 100
LN_EPS = 1e-5


# ---------------------------------------------------------------------------
# Host-side constant construction
# ---------------------------------------------------------------------------

def _vert_matrix():
    """V: out = V @ F gives 0.25*(F[i-1]+F[i+1]) with reflect at the edges."""
    V = np.zeros((M, M), np.float32)
    for i in range(M):
        up = 1 if i == 0 else i - 1
        dn = M - 2 if i == M - 1 else i + 1
        V[i, up] += 0.25
        V[i, dn] += 0.25
    return V


def _conv1_mats(w1):
    """A[dx][yb] [128,128]: column (oc*8+ys) holds w1[oc, dy, dx] at input
    row r = 2*(8*yb+ys)+dy-2 (rows outside [0,128) dropped = zero pad)."""
    w1 = np.asarray(w1, np.float32)
    mats = []
    for dx in range(5):
        per_yb = []
        for yb in range(8):
            A = np.zeros((M, 128), np.float32)
            for oc in range(16):
                for ys in range(8):
                    y = 8 * yb + ys
                    for dy in range(5):
                        r = 2 * y + dy - 2
                        if 0 <= r < M:
                            A[r, oc * 8 + ys] = w1[oc, 0, dy, dx]
            per_yb.append(A)
        mats.append(per_yb)
    return mats


def _conv_mats(w, ich, och, bsz):
    """Banded mats for 3x3 s2 p1 conv blocked by row groups of size bsz(out).
    Input row groups have size 2*bsz. k = ic*(2*bsz) + o, m = oc*bsz + ys,
    input row offset o = 2*ys + dy - 1. Returns cur[dx], prev[dx]."""
    w = np.asarray(w, np.float32)
    cur, prev = [], []
    for dx in range(3):
        Bc = np.zeros((128, 128), np.float32)
        Bp = np.zeros((128, 128), np.float32)
        for oc in range(och):
            for ic in range(ich):
                for ys in range(bsz):
                    for dy in range(3):
                        o = 2 * ys + dy - 1
                        if 0 <= o < 2 * bsz:
                            Bc[ic * 2 * bsz + o, oc * bsz + ys] = w[oc, ic, dy, dx]
                        elif o == -1:
                            Bp[ic * 2 * bsz + 2 * bsz - 1, oc * bsz + ys] = \
                                w[oc, ic, dy, dx]
        cur.append(Bc)
        prev.append(Bp)
    return cur, prev


def make_consts(w1, b1, w2, b2, w3, b3, wl, bl):
    c = {}
    V = _vert_matrix()
    c["vqT"] = np.ascontiguousarray(V.T)
    c["iq"] = (0.25 * np.eye(M)).astype(np.float32)
    c["ones128"] = np.ones((M, M), np.float32)
    c["ones_norm"] = np.full((M, M), 1.0 / (M * M), np.float32)
    a1 = _conv1_mats(w1)
    for dx in range(5):
        for yb in range(8):
            c[f"a1_{dx}_{yb}"] = a1[dx][yb]
    c2c, c2p = _conv_mats(w2, 16, 32, 4)
    c3c, c3p = _conv_mats(w3, 32, 64, 2)
    for dx in range(3):
        c[f"b2c_{dx}"] = c2c[dx]
        c[f"b2p_{dx}"] = c2p[dx]
        c[f"b3c_{dx}"] = c3c[dx]
        c[f"b3p_{dx}"] = c3p[dx]
    sel = np.zeros((128, 64), np.float32)
    for k in range(128):
        sel[k, k // 2] = 1.0 / 256.0
    c["sel"] = sel
    c["wlT"] = np.ascontiguousarray(np.asarray(wl, np.float32).T)  # [64,5]
    c["b1e"] = np.asarray(b1, np.float32).repeat(8).reshape(128, 1)
    c["b2e"] = np.asarray(b2, np.float32).repeat(4).reshape(128, 1)
    c["b3e"] = np.asarray(b3, np.float32).repeat(2).reshape(128, 1)
    c["ble"] = np.asarray(bl, np.float32).reshape(5, 1)
    return c


def _xslice(dx, pad, w_in, w_out):
    """For out col xo, input col is 2*xo+dx-pad; clip to valid input range.
    Returns (src_lo, src_hi_excl, dst_lo, dst_hi_excl)."""
    xs = [(2 * xo + dx - pad, xo) for xo in range(w_out)
          if 0 <= 2 * xo + dx - pad < w_in]
    return xs[0][0], xs[-1][0] + 1, xs[0][1], xs[-1][1] + 1


# ---------------------------------------------------------------------------
# Device program
# ---------------------------------------------------------------------------

def build_program(img=IMG, iters=NUM_ITERS, bi=16, ci=8, sim_gelu=False):
    nblk = img // bi
    ncblk = img // ci
    nc = bacc.Bacc("TRN2", target_bir_lowering=False, debug=False,
                   enable_asserts=False)

    dram = {}

    def din(name, shape):
        dram[name] = nc.dram_tensor(name, list(shape), F32,
                                    kind="ExternalInput").ap()
        return dram[name]

    sst = din("sst", (img, M, M))
    din("lnw", (M, M))
    din("lnb", (M, M))
    for name, shape in [("vqT", (M, M)), ("iq", (M, M)), ("ones128", (M, M)),
                        ("ones_norm", (M, M)), ("sel", (128, 64)),
                        ("wlT", (64, 5)), ("b1e", (128, 1)), ("b2e", (128, 1)),
                        ("b3e", (128, 1)), ("ble", (5, 1))]:
        din(name, shape)
    for dx in range(5):
        for yb in range(8):
            din(f"a1_{dx}_{yb}", (M, 128))
    for dx in range(3):
        for s in ("b2c", "b2p", "b3c", "b3p"):
            din(f"{s}_{dx}", (128, 128))
    out5_d = nc.dram_tensor("out5", [5, img], F32, kind="ExternalOutput").ap()

    with tile.TileContext(nc) as tc, ExitStack() as ctx:
        consts = ctx.enter_context(tc.tile_pool(name="consts", bufs=1))
        stats = ctx.enter_context(tc.tile_pool(name="stats", bufs=1))

        sb = {}
        for name, t in dram.items():
            if name == "sst":
                continue
            sb[name] = consts.tile(list(t.shape), F32, tag=name, name=name)
            nc.sync.dma_start(sb[name][:], t[:])

        def flat(ap3):  # [128, bi, M] -> [128, bi*M]
            return ap3.rearrange("p a b -> p (a b)")

        zero1 = consts.tile([M, 1], F32, tag="zero1", name="zero1")
        eps1 = consts.tile([M, 1], F32, tag="eps1", name="eps1")
        nc.vector.memset(zero1[:], 0.0)
        nc.vector.memset(eps1[:], LN_EPS)

        with tc.tile_pool(name="pF", bufs=1) as pF:
            F = [pF.tile([M, bi, M], F32, tag=f"F{b}", name=f"F{b}") for b in range(nblk)]
            for b in range(nblk):
                nc.sync.dma_start(
                    F[b][:],
                    sst[b * bi:(b + 1) * bi].rearrange("i p c -> p i c"))

            with tc.tile_pool(name="pmask", bufs=1) as pmask:
                mask = [pmask.tile([M, bi, M], U8, tag=f"mk{b}", name=f"mk{b}")
                        for b in range(nblk)]

                # ================= init: nanmean row/col fill ================
                with tc.tile_pool(name="pinit", bufs=1) as pinit, \
                     tc.tile_pool(name="psin", bufs=1, space="PSUM") as psin:
                    rs = stats.tile([M, img], F32, tag="rs")
                    rcn = stats.tile([M, img], F32, tag="rcn")
                    rmh = stats.tile([M, img], F32, tag="rmh")
                    zero_b = bass.AP(tensor=zero1[:].tensor,
                                     offset=zero1[:].offset,
                                     ap=[zero1[:].ap[0], [0, bi * M]])

                    for b in range(nblk):
                        isl = slice(b * bi, (b + 1) * bi)
                        maskf = pinit.tile([M, bi, M], F32, tag="mf",
                                           name=f"mf{b}")
                        nc.vector.tensor_tensor(maskf[:], F[b][:], F[b][:],
                                                op=OP.not_equal)
                        nc.vector.tensor_copy(mask[b][:], maskf[:])
                        nc.vector.copy_predicated(flat(F[b][:]), flat(mask[b][:]), zero_b)
                        nc.vector.reduce_sum(rs[:, isl], F[b][:],
                                             axis=mybir.AxisListType.X)
                        nc.vector.reduce_sum(rcn[:, isl], maskf[:],
                                             axis=mybir.AxisListType.X)

                    # rmh = 0.5 * rowsum / (128 - rowcnt)
                    nc.vector.tensor_scalar(rmh[:], rcn[:], -1.0, 128.0,
                                            op0=OP.mult, op1=OP.add)
                    nc.vector.reciprocal(rmh[:], rmh[:])
                    nc.vector.scalar_tensor_tensor(rmh[:], rs[:], 0.5, rmh[:],
                                                   op0=OP.mult, op1=OP.mult)

                    for b in range(nblk):
                        mfl = pinit.tile([M, bi, M], F32, tag="mf",
                                         name=f"mfl{b}")
                        nc.vector.tensor_copy(mfl[:], mask[b][:])
                        cs = psin.tile([M, bi * M], F32, tag="cs")
                        cc = psin.tile([M, bi * M], F32, tag="cc")
                        fb = flat(F[b][:])
                        mb = flat(mfl[:])
                        for s in range(0, bi * M, 512):
                            nc.tensor.matmul(cs[:, s:s + 512], sb["ones128"][:],
                                             fb[:, s:s + 512],
                                             start=True, stop=True)
                            nc.tensor.matmul(cc[:, s:s + 512], sb["ones128"][:],
                                             mb[:, s:s + 512],
                                             start=True, stop=True)
                        td = pinit.tile([M, bi * M], F32, tag="td")
                        rc = pinit.tile([M, bi, M], F32, tag="rc")
                        nc.vector.tensor_scalar(td[:], cc[:], -1.0, 128.0,
                                                op0=OP.mult, op1=OP.add)
                        nc.vector.reciprocal(td[:], td[:])
                        nc.vector.scalar_tensor_tensor(td[:], cs[:], 0.5, td[:],
                                                       op0=OP.mult, op1=OP.mult)
                        tdv = td[:].rearrange("p (a b) -> p a b", a=bi)
                        for ii in range(bi):
                            i = b * bi + ii
                            nc.scalar.activation(rc[:, ii, :], tdv[:, ii, :],
                                                 AF.Identity,
                                                 bias=rmh[:, i:i + 1],
                                                 scale=1.0)
                        nc.vector.copy_predicated(F[b][:], mask[b][:], rc[:])

                # ================= Jacobi iterations =================
                with tc.tile_pool(name="ph", bufs=1) as ph, \
                     tc.tile_pool(name="psj", bufs=2, space="PSUM") as psj:
                    H = [ph.tile([M, bi, M], F32, tag=f"h{b}", name=f"h{b}")
                         for b in range(nblk)]
                    for _ in range(iters):
                        for b in range(nblk):
                            fb = flat(F[b][:])
                            hb = flat(H[b][:])
                            n = bi * M
                            nc.vector.tensor_tensor(hb[:, 1:n - 1],
                                                    fb[:, 0:n - 2],
                                                    fb[:, 2:n], op=OP.add)
                            # reflect: col0 = 2*col1, col127 = 2*col126
                            nc.vector.tensor_scalar_mul(
                                H[b][:, :, 0:M:M - 1],
                                F[b][:, :, 1:M - 1:M - 3], 2.0)
                        for b in range(nblk):
                            fb = flat(F[b][:])
                            hb = flat(H[b][:])
                            ps = psj.tile([M, bi * M], F32, tag="jac")
                            for s in range(0, bi * M, 512):
                                nc.tensor.matmul(ps[:, s:s + 512], sb["vqT"][:],
                                                 fb[:, s:s + 512],
                                                 start=True, stop=False)
                                nc.tensor.matmul(ps[:, s:s + 512], sb["iq"][:],
                                                 hb[:, s:s + 512],
                                                 start=False, stop=True)
                            nc.vector.copy_predicated(flat(F[b][:]),
                                                      flat(mask[b][:]), ps[:])

            # ================= LayerNorm =================
            with tc.tile_pool(name="pconv", bufs=1) as pc:
                s1 = stats.tile([M, img], F32, tag="s1")
                s2 = stats.tile([M, img], F32, tag="s2")
                sq = pc.tile([M, bi * M], F32, tag="sq")
                for b in range(nblk):
                    isl = slice(b * bi, (b + 1) * bi)
                    nc.vector.reduce_sum(s1[:, isl], F[b][:],
                                         axis=mybir.AxisListType.X)
                    nc.scalar.activation(sq[:], flat(F[b][:]), AF.Square, bias=zero1[:])
                    nc.vector.reduce_sum(
                        s2[:, isl],
                        sq[:].rearrange("p (a b) -> p a b", a=bi),
                        axis=mybir.AxisListType.X)

                var = stats.tile([M, img], F32, tag="var")
                rstd = stats.tile([M, img], F32, tag="rstd")
                nbias = stats.tile([M, img], F32, tag="nbias")
                with tc.tile_pool(name="psln", bufs=1, space="PSUM") as psln:
                    mu = psln.tile([M, img], F32, tag="mu")
                    ms = psln.tile([M, img], F32, tag="ms")
                    nc.tensor.matmul(mu[:], sb["ones_norm"][:], s1[:],
                                     start=True, stop=True)
                    nc.tensor.matmul(ms[:], sb["ones_norm"][:], s2[:],
                                     start=True, stop=True)
                    mu_sb = stats.tile([M, img], F32, tag="musb")
                    nc.scalar.copy(mu_sb[:], mu[:])
                    nc.vector.tensor_tensor(var[:], mu_sb[:], mu_sb[:],
                                            op=OP.mult)
                    nc.vector.tensor_tensor(var[:], ms[:], var[:],
                                            op=OP.subtract)
                    nc.scalar.activation(rstd[:], var[:], AF.Sqrt, bias=eps1[:])
                    nc.vector.reciprocal(rstd[:], rstd[:])
                    nc.vector.scalar_tensor_tensor(nbias[:], mu_sb[:], -1.0,
                                                   rstd[:], op0=OP.mult,
                                                   op1=OP.mult)
                # in-place: F <- (F - mu)*rstd, then *ln_w + ln_b
                for b in range(nblk):
                    for ii in range(bi):
                        i = b * bi + ii
                        nc.scalar.activation(F[b][:, ii, :], F[b][:, ii, :],
                                             AF.Identity,
                                             bias=nbias[:, i:i + 1],
                                             scale=rstd[:, i:i + 1])
                lnw_ap = sb["lnw"][:]
                lnb_ap = sb["lnb"][:]
                lnw_b = bass.AP(tensor=lnw_ap.tensor, offset=lnw_ap.offset,
                                ap=[lnw_ap.ap[0], [0, bi], lnw_ap.ap[1]])
                lnb_b = bass.AP(tensor=lnb_ap.tensor, offset=lnb_ap.offset,
                                ap=[lnb_ap.ap[0], [0, bi], lnb_ap.ap[1]])
                for b in range(nblk):
                    nc.vector.tensor_tensor(F[b][:], F[b][:], lnw_b,
                                            op=OP.mult)
                    nc.vector.tensor_tensor(F[b][:], F[b][:], lnb_b,
                                            op=OP.add)

                # ================= conv stack =================
                # padded inputs: every kernel-x tap reads a full-width
                # stride-2 slice, so every matmul writes a full, contiguous
                # (img,x) range = one PSUM bank per row-group.
                xp = pc.tile([128, ci, 131], F32, tag="xp")    # pad L2, R1
                v1 = pc.tile([128, ci, 8, 66], F32, tag="v1")  # (img, yb, 1+64+1)
                v2 = pc.tile([128, ci, 8, 34], F32, tag="v2")  # (img, c, 1+32+1)
                v3 = pc.tile([128, ci, 8, 16], F32, tag="v3")  # (img, c, x)
                pall = stats.tile([128, img], F32, tag="pall")
                nc.vector.memset(xp[:], 0.0)
                nc.vector.memset(v1[:], 0.0)
                nc.vector.memset(v2[:], 0.0)

                def drain3(ap2, n):  # [128, ci*n] -> [128, ci, n]
                    return ap2.rearrange("p (i x) -> p i x", i=ci)

                gt1 = pc.tile([128, ci * 64], F32, tag="gt1")
                gt2 = pc.tile([128, ci * 64], F32, tag="gt2")

                def gelu_drain(out_ap, in_ap, bias_ap, n):
                    if not sim_gelu:
                        nc.scalar.activation(out_ap, in_ap,
                                             AF.Gelu_apprx_tanh, bias=bias_ap)
                        return
                    # sim fallback: 0.5*t*(1+tanh(c*(t + 0.044715 t^3)))
                    c0, a0 = 0.7978845608028654, 0.044715
                    t = gt1[:, :n]
                    u = gt2[:, :n]
                    nc.scalar.activation(t, in_ap, AF.Identity, bias=bias_ap)
                    nc.scalar.activation(u, t, AF.Square, bias=zero1[:])
                    nc.vector.tensor_scalar(u, u, a0, 1.0, op0=OP.mult,
                                            op1=OP.add)
                    nc.vector.scalar_tensor_tensor(u, t, c0, u, op0=OP.mult,
                                                   op1=OP.mult)
                    nc.scalar.activation(u, u, AF.Tanh, bias=zero1[:])
                    nc.vector.scalar_tensor_tensor(u, u, 1.0, t, op0=OP.add,
                                                   op1=OP.mult)
                    nc.vector.tensor_scalar_mul(out_ap, u, 0.5)

                with tc.tile_pool(name="psc", bufs=1, space="PSUM") as psc:
                    for cb in range(ncblk):
                        fb = cb * ci // bi
                        io = (cb * ci) % bi
                        nc.vector.tensor_copy(xp[:, :, 2:130],
                                              F[fb][:, io:io + ci, :])

                        # conv1: 5x5 s2 p2 -> [16ch, 64y, 64x]
                        ps1 = psc.tile([128, 8, ci * 64], F32, tag="cps")
                        for dx in range(5):
                            for yb in range(8):
                                nc.tensor.matmul(
                                    ps1[:, yb, :], sb[f"a1_{dx}_{yb}"][:],
                                    xp[:, :, dx:dx + 127:2],
                                    start=(dx == 0), stop=(dx == 4))
                        for yb in range(8):
                            gelu_drain(v1[:, :, yb, 1:65],
                                       drain3(ps1[:, yb, :], 64),
                                       sb["b1e"][:], ci * 64)

                        # conv2: 3x3 s2 p1 -> [32ch, 32y, 32x]
                        # taps: (mat, src row-block delta, start, stop)
                        ps2 = psc.tile([128, 8, 512], F32, tag="cps")
                        taps2 = [("b2c_1", 0), ("b2c_0", 0), ("b2p_0", -1),
                                 ("b2p_1", -1), ("b2p_2", -1), ("b2c_2", 0)]
                        for ti, (mat, dc) in enumerate(taps2):
                            dx = int(mat[-1])
                            for c in range(8):
                                if c + dc < 0:
                                    continue
                                nc.tensor.matmul(
                                    ps2[:, c, 0:ci * 32], sb[mat][:],
                                    v1[:, :, c + dc, dx:dx + 63:2],
                                    start=(ti == 0), stop=(ti == len(taps2) - 1))
                        for c in range(8):
                            gelu_drain(v2[:, :, c, 1:33],
                                       drain3(ps2[:, c, 0:ci * 32], 32),
                                       sb["b2e"][:], ci * 32)

                        # conv3: 3x3 s2 p1 -> [64ch, 16y, 16x]
                        ps3 = psc.tile([128, 8, 512], F32, tag="cps")
                        taps3 = [("b3c_1", 0), ("b3c_0", 0), ("b3p_0", -1),
                                 ("b3p_1", -1), ("b3p_2", -1), ("b3c_2", 0)]
                        for ti, (mat, dc) in enumerate(taps3):
                            dx = int(mat[-1])
                            for c in range(8):
                                if c + dc < 0:
                                    continue
                                nc.tensor.matmul(
                                    ps3[:, c, 0:ci * 16], sb[mat][:],
                                    v2[:, :, c + dc, dx:dx + 31:2],
                                    start=(ti == 0), stop=(ti == len(taps3) - 1))
                        for c in range(8):
                            gelu_drain(v3[:, :, c, :],
                                       drain3(ps3[:, c, 0:ci * 16], 16),
                                       sb["b3e"][:], ci * 16)

                        nc.vector.reduce_sum(
                            pall[:, cb * ci:(cb + 1) * ci],
                            v3[:].rearrange("p i c x -> p i (c x)"),
                            axis=mybir.AxisListType.X)

                    # head
                    feat_ps = psc.tile([64, img], F32, tag="cps")
                    nc.tensor.matmul(feat_ps[:], sb["sel"][:], pall[:],
                                     start=True, stop=True)
                    feat = stats.tile([64, img], F32, tag="feat")
                    nc.scalar.copy(feat[:], feat_ps[:])
                    o5_ps = psc.tile([5, img], F32, tag="cps")
                    nc.tensor.matmul(o5_ps[:], sb["wlT"][:], feat[:],
                                     start=True, stop=True)
                    o5 = stats.tile([5, img], F32, tag="o5")
                    nc.scalar.activation(o5[:], o5_ps[:], AF.Identity,
                                         bias=sb["ble"][:])
                    nc.sync.dma_start(out5_d[:], o5[:])

    nc.compile()
    return nc


# ---------------------------------------------------------------------------
# Host entry point
# ---------------------------------------------------------------------------

def make_in_maps(sst, ln_w, ln_b, w1, b1, w2, b2, w3, b3, wl, bl,
                 ncores=NCORES, img=IMG):
    sst = np.asarray(sst, np.float32)
    base = {k: np.ascontiguousarray(v, np.float32)
            for k, v in make_consts(w1, b1, w2, b2, w3, b3, wl, bl).items()}
    base["lnw"] = np.ascontiguousarray(ln_w, np.float32)
    base["lnb"] = np.ascontiguousarray(ln_b, np.float32)
    in_maps = []
    for c in range(ncores):
        m = dict(base)
        m["sst"] = np.ascontiguousarray(sst[c * img:(c + 1) * img])
        in_maps.append(m)
    return in_maps


def assemble_outputs(out5):
    """out5 [B, 5] -> (mu [B,2], sigma [B,2,2]) as in the reference."""
    mu_y, mu_x = out5[:, 0], out5[:, 1]
    sig_y, sig_x, sig_yx = out5[:, 2], out5[:, 3], out5[:, 4]
    mu = np.stack([mu_x, mu_y], axis=1).astype(np.float32)
    sigma = np.zeros((out5.shape[0], 2, 2), np.float32)
    sigma[:, 0, 0] = np.exp(sig_y)
    sigma[:, 1, 1] = np.exp(sig_x)
    sigma[:, 0, 1] = sig_yx
    sigma[:, 1, 0] = sig_yx
    mu = np.nan_to_num(mu, nan=0.0, posinf=0.0, neginf=0.0)
    sigma = np.nan_to_num(sigma, nan=1.0, posinf=1.0, neginf=1.0)
    return mu, sigma


_CACHE = {}


def kernel(sst, ln_w, ln_b, w1, b1, w2, b2, w3, b3, wl, bl, trace=False):
    in_maps = make_in_maps(sst, ln_w, ln_b, w1, b1, w2, b2, w3, b3, wl, bl)
    if "nc" not in _CACHE:
        _CACHE["nc"] = build_program()
    res = bass_utils.run_bass_kernel_spmd(_CACHE["nc"], in_maps,
                                          core_ids=list(range(NCORES)),
                                          trace=trace)
    _CACHE["last_result"] = res
    out5 = np.concatenate([res.results[c]["out5"].T for c in range(NCORES)],
                          axis=0)
    return assemble_outputs(out5)
